# revision 1
# baseline (speedup 1.0000x reference)
"""TRN2 Bass kernel for nn_Block_58574763983799 (dense transformer block).

Self-contained: builds/compiles the Bass program on first call (cached),
shards the batch (data-parallel) over 8 NeuronCores, computes in fp16
(fp32 accumulation in PSUM; layernorm statistics and softmax in fp32),
returns fp32.

  kernel(**inputs) -> np.ndarray [2048, 64, 384] float32
"""
import sys
if "/opt/trn_rl_repo" not in sys.path:
    sys.path.insert(0, "/opt/trn_rl_repo")
import numpy as np
import ml_dtypes

import concourse.bass as bass
import concourse.mybir as mybir
import concourse.tile as tile
from contextlib import ExitStack
from concourse.vector_clock import ScopedClock, VectorClock

F32 = mybir.dt.float32
BF16 = mybir.dt.float16  # 16-bit compute dtype (fp16: need mantissa for softmax exponent accuracy)
AF = mybir.ActivationFunctionType
ALU = mybir.AluOpType

C = 384
H = 8
D = 48
DP = 64          # padded head dim
T = 64           # tokens per row (attention block)
FF = 4 * C       # 1536
CP = H * DP      # 512 padded concat dim
RP = 128         # rowpair tokens
ST = 512         # supertile tokens
NEG = -1.0e9
SCALE = D * (C ** -0.5)
EPS = 1e-5
MAGIC = 0x5F3759DF


def patch_drain():
    """Split the TileContext tail-drain's semaphore waits into 1-wait NOPs.

    This walrus build rejects >2 sync-wait commands on SP CTRL instructions
    ("Too many sync wait commands"), and the stock tail drain carries one wait
    per live processor.  Chain single-wait NOPs on SP instead; program order on
    the SP sequencer then makes the drain itself wait-free.
    """
    if getattr(tile.TileContext, "_drain_patched", False):
        return

    def _drain_and_barrier(self, tick_clock, wait_clock):
        nc = self.nc
        gcv = tick_clock.global_clock
        ticks = list(gcv)
        nz = [(i, t) for i, t in enumerate(ticks) if t > 0]
        for i, t in nz:
            vec = [0] * len(ticks)
            vec[i] = t
            nop = nc.sync.nop()
            wait_clock.add_sem_waits(nop.ins, ScopedClock({None: VectorClock(vec)}))
        nc.sync.drain()
        nc.all_engine_barrier()
        assert self.sems is not None
        popped = nc._tile_sem_poison_stack.pop()
        assert popped is self._sem_poison
        nc.clear_and_free_semaphores(list(self.sems.allocated().values()))
        nc.all_engine_barrier()

    tile.TileContext._drain_and_barrier = _drain_and_barrier
    tile.TileContext._drain_patched = True


def build_weights(inputs):
    """Host-side weight prep from the raw reference inputs (numpy f32)."""
    Wq = np.asarray(inputs["Wq"], np.float32)  # [H, C, D]
    Wk = np.asarray(inputs["Wk"], np.float32)
    Wv = np.asarray(inputs["Wv"], np.float32)
    Wp = np.asarray(inputs["Wp"], np.float32)  # [C, C]
    W1 = np.asarray(inputs["W1"], np.float32)  # [C, FF]
    W2 = np.asarray(inputs["W2"], np.float32)  # [FF, C]

    bf = np.float16
    # wqk [C, 2*CP]: cols 64h..64h+47 = Wq_h * SCALE (scale folded into q)
    wqk = np.zeros((C, 2 * CP), np.float32)
    for h in range(H):
        wqk[:, DP * h : DP * h + D] = Wq[h] * SCALE
        wqk[:, CP + DP * h : CP + DP * h + D] = Wk[h]
    # wv [C, CP]
    wv = np.zeros((C, CP), np.float32)
    for h in range(H):
        wv[:, DP * h : DP * h + D] = Wv[h]
    # wp [CP, C] padded rows
    wp = np.zeros((CP, C), np.float32)
    for h in range(H):
        wp[DP * h : DP * h + D, :] = Wp[D * h : D * h + D, :]

    # block-causal additive mask [RP, RP] (2 diag 64-blocks, causal within)
    madd = np.full((RP, RP), NEG, np.float32)
    for b in range(2):
        for tl in range(T):
            madd[b * T + tl, b * T : b * T + tl + 1] = 0.0

    ident = np.eye(128, dtype=np.float32)

    return {
        "wqk": wqk.astype(bf),
        "wv": wv.astype(bf),
        "wp": wp.astype(bf),
        "w1": W1.astype(bf),
        "w2": W2.astype(bf),
        "madd": madd,
        "ident": ident.astype(bf),
        "bp": np.asarray(inputs["bp"], np.float32),
        "b1": np.asarray(inputs["b1"], np.float32),
        "b2": np.asarray(inputs["b2"], np.float32),
        "ln1g": np.asarray(inputs["ln1_g"], np.float32),
        "ln1b": np.asarray(inputs["ln1_b"], np.float32),
        "ln2g": np.asarray(inputs["ln2_g"], np.float32),
        "ln2b": np.asarray(inputs["ln2_b"], np.float32),
    }


def build_nc(n_tok, trivial_ln=True, trivial_bias=True, debug=False):
    """Build the Bass program for one core processing [n_tok, C] tokens."""
    patch_drain()
    assert n_tok % ST == 0
    n_st = n_tok // ST
    nc = bass.Bass()

    x_d = nc.dram_tensor("x", [n_tok, C], BF16, kind="ExternalInput")
    wqk_d = nc.dram_tensor("wqk", [C, 2 * CP], BF16, kind="ExternalInput")
    wv_d = nc.dram_tensor("wv", [C, CP], BF16, kind="ExternalInput")
    wp_d = nc.dram_tensor("wp", [CP, C], BF16, kind="ExternalInput")
    w1_d = nc.dram_tensor("w1", [C, FF], BF16, kind="ExternalInput")
    w2_d = nc.dram_tensor("w2", [FF, C], BF16, kind="ExternalInput")
    madd_d = nc.dram_tensor("madd", [RP, RP], F32, kind="ExternalInput")
    ident_d = nc.dram_tensor("ident", [128, 128], BF16, kind="ExternalInput")
    bp_d = nc.dram_tensor("bp", [C], F32, kind="ExternalInput")
    b1_d = nc.dram_tensor("b1", [FF], F32, kind="ExternalInput")
    b2_d = nc.dram_tensor("b2", [C], F32, kind="ExternalInput")
    if not trivial_ln:
        ln1g_d = nc.dram_tensor("ln1g", [C], F32, kind="ExternalInput")
        ln1b_d = nc.dram_tensor("ln1b", [C], F32, kind="ExternalInput")
        ln2g_d = nc.dram_tensor("ln2g", [C], F32, kind="ExternalInput")
        ln2b_d = nc.dram_tensor("ln2b", [C], F32, kind="ExternalInput")
    out_d = nc.dram_tensor("out", [n_tok, C], BF16, kind="ExternalOutput")
    dbg = {}
    if debug:
        for nm, shp in (("h1T", [C, n_tok]), ("qk", [2 * CP, n_tok]),
                        ("v", [n_tok, CP]), ("wei", [n_tok, RP]),
                        ("attn", [CP, n_tok]), ("x2", [n_tok, C]),
                        ("relu", [FF, n_tok])):
            dbg[nm] = nc.dram_tensor("dbg_" + nm, shp, BF16, kind="ExternalOutput")

    with tile.TileContext(nc) as tc, ExitStack() as ctx:
        cpool = ctx.enter_context(tc.tile_pool(name="consts", bufs=1))

        # ---- constants into SBUF ----
        wqk_sb = cpool.tile([128, 3, 2 * CP], BF16)
        nc.gpsimd.dma_start(out=wqk_sb, in_=wqk_d.rearrange("(a p) n -> p a n", p=128))
        wv_sb = cpool.tile([128, 3, CP], BF16)
        nc.gpsimd.dma_start(out=wv_sb, in_=wv_d.rearrange("(a p) n -> p a n", p=128))
        wp_sb = cpool.tile([128, 4, C], BF16)
        nc.gpsimd.dma_start(out=wp_sb, in_=wp_d.rearrange("(a p) n -> p a n", p=128))
        w1_sb = cpool.tile([128, 3, FF], BF16)
        nc.gpsimd.dma_start(out=w1_sb, in_=w1_d.rearrange("(a p) n -> p a n", p=128))
        w2_sb = cpool.tile([128, 12, C], BF16)
        nc.gpsimd.dma_start(out=w2_sb, in_=w2_d.rearrange("(a p) n -> p a n", p=128))
        madd_sb = cpool.tile([128, RP], F32)
        nc.gpsimd.dma_start(out=madd_sb, in_=madd_d[:, :])
        ident_sb = cpool.tile([128, 128], BF16)
        nc.gpsimd.dma_start(out=ident_sb, in_=ident_d[:, :])
        b1_sb = cpool.tile([128, 12], F32)
        nc.gpsimd.dma_start(out=b1_sb, in_=b1_d.rearrange("(a p) -> p a", p=128))
        b2_sb = cpool.tile([128, 3], F32)
        nc.gpsimd.dma_start(out=b2_sb, in_=b2_d.rearrange("(a p) -> p a", p=128))
        bpb_sb = cpool.tile([128, C], BF16)
        nc.gpsimd.dma_start(out=bpb_sb, in_=bp_d[None, :].to_broadcast([128, C]))
        magic_sb = cpool.tile([128, 4], mybir.dt.uint32)
        nc.vector.memset(magic_sb, MAGIC)
        csub_sb = cpool.tile([128, 1], F32)
        nc.vector.memset(csub_sb, -40.0)
        if not trivial_ln:
            lnb = {}
            for nm, dten in (("ln1g", ln1g_d), ("ln1b", ln1b_d),
                             ("ln2g", ln2g_d), ("ln2b", ln2b_d)):
                t_ = cpool.tile([128, C], BF16, tag=nm)
                nc.gpsimd.dma_start(out=t_, in_=dten[None, :].to_broadcast([128, C]))
                lnb[nm] = t_

        # ---- pools ----
        xin = ctx.enter_context(tc.tile_pool(name="xin", bufs=12))
        stat = ctx.enter_context(tc.tile_pool(name="stat", bufs=4))
        hbuf = ctx.enter_context(tc.tile_pool(name="hbuf", bufs=8))
        htp = ctx.enter_context(tc.tile_pool(name="htp", bufs=2))
        qkp = ctx.enter_context(tc.tile_pool(name="qkp", bufs=3, space="PSUM"))
        ffp = ctx.enter_context(tc.tile_pool(name="ffp", bufs=2, space="PSUM"))
        qks = ctx.enter_context(tc.tile_pool(name="qks", bufs=2))
        vsb = ctx.enter_context(tc.tile_pool(name="vsb", bufs=8))
        scp = ctx.enter_context(tc.tile_pool(name="scp", bufs=2, space="PSUM"))
        smx = ctx.enter_context(tc.tile_pool(name="smx", bufs=6))
        smv = ctx.enter_context(tc.tile_pool(name="smv", bufs=12))
        atp = ctx.enter_context(tc.tile_pool(name="atp", bufs=1, space="PSUM"))
        ats = ctx.enter_context(tc.tile_pool(name="ats", bufs=2))
        x2p = ctx.enter_context(tc.tile_pool(name="x2p", bufs=8))
        rlu = ctx.enter_context(tc.tile_pool(name="rlu", bufs=2))
        f2s = ctx.enter_context(tc.tile_pool(name="f2s", bufs=2))
        oub = ctx.enter_context(tc.tile_pool(name="oub", bufs=8))

        x_v = x_d.rearrange("(s p) c -> s p c", p=RP)      # [n_rp, 128, C]
        out_v = out_d.rearrange("(s p) c -> s p c", p=RP)

        def layer_norm(src_tiles, gname, bname):
            """src_tiles: 4 SBUF [128, C] bf16 tiles -> returns 4 h tiles bf16."""
            mv = stat.tile([128, 4, 2], F32, tag="mv", name="mv")
            for rp in range(4):
                bstat = stat.tile([128, 6], F32, tag="bstat", name="bstat")
                nc.vector.bn_stats(out=bstat, in_=src_tiles[rp])
                nc.vector.bn_aggr(out=mv[:, rp, :], in_=bstat)
            # rstd = rsqrt(var + eps), batched Newton on [128, 4]
            ve = stat.tile([128, 4], F32, tag="ve", name="ve")
            nc.vector.tensor_scalar_add(ve, mv[:, :, 1], EPS)
            vh = stat.tile([128, 4], F32, tag="vh", name="vh")
            nc.vector.tensor_scalar_mul(vh, ve, -0.5)
            ub = stat.tile([128, 4], mybir.dt.uint32, tag="ub", name="ub")
            nc.vector.tensor_scalar(
                out=ub, in0=ve.bitcast(mybir.dt.uint32), scalar1=1,
                scalar2=None, op0=ALU.logical_shift_right)
            y = stat.tile([128, 4], F32, tag="y", name="y")
            nc.vector.tensor_tensor(
                out=y.bitcast(mybir.dt.uint32), in0=magic_sb, in1=ub,
                op=ALU.subtract)
            tq = stat.tile([128, 4], F32, tag="tq", name="tq")
            for _ in range(2):
                nc.vector.tensor_tensor(out=tq, in0=y, in1=y, op=ALU.mult)
                nc.vector.tensor_tensor(out=tq, in0=tq, in1=vh, op=ALU.mult)
                nc.vector.tensor_scalar_add(tq, tq, 1.5)
                nc.vector.tensor_tensor(out=y, in0=y, in1=tq, op=ALU.mult)
            hs = []
            for rp in range(4):
                h_ = hbuf.tile([128, C], BF16, tag="h", name="h")
                nc.vector.tensor_scalar(
                    out=h_, in0=src_tiles[rp],
                    scalar1=mv[:, rp, 0:1], scalar2=y[:, rp : rp + 1],
                    op0=ALU.subtract, op1=ALU.mult)
                if not trivial_ln:
                    nc.vector.tensor_tensor(out=h_, in0=h_, in1=lnb[gname], op=ALU.mult)
                    nc.vector.tensor_tensor(out=h_, in0=h_, in1=lnb[bname], op=ALU.add)
                hs.append(h_)
            return hs

        def transpose_sb(h_tiles, tag):
            """4x [128, C] bf16 -> hT 3x [128, ST] bf16 via DMA-transpose."""
            hT = [htp.tile([128, ST], BF16, tag=f"{tag}{ct}", name=f"{tag}{ct}") for ct in range(3)]
            for rp in range(4):
                for ct in range(3):
                    nc.sync.dma_start(
                        out=hT[ct][:, rp * RP : (rp + 1) * RP],
                        in_=h_tiles[rp][:, ct * 128 : (ct + 1) * 128],
                        transpose=True)
            return hT

        xq = {}

        def load_x(st):
            x_t = []
            for rp in range(4):
                xt = xin.tile([RP, C], BF16, tag="x", name="x")
                nc.gpsimd.dma_start(out=xt, in_=x_v[st * 4 + rp])
                x_t.append(xt)
            xq[st] = x_t

        def front_ln(st):
            if st not in xq:
                load_x(st)
            if st + 1 < n_st:
                load_x(st + 1)
            x_t = xq.pop(st)

            h1 = layer_norm(x_t, "ln1g", "ln1b")
            h1T = transpose_sb(h1, "h1T")

            if trivial_bias:
                xpb = x_t
            else:
                xpb = []
                for rp in range(4):
                    xp_ = xin.tile([RP, C], BF16, tag="xpb", name="xpb")
                    nc.vector.tensor_tensor(out=xp_, in0=x_t[rp], in1=bpb_sb, op=ALU.add)
                    xpb.append(xp_)
            return dict(x_t=x_t, xpb=xpb, h1T=h1T)

        def front_mm(st, S):
            h1T = S["h1T"]
            qk_sb = []
            for mt in range(8):
                ps = qkp.tile([128, ST], F32, tag="qkps", name="qkps")
                for kt in range(3):
                    nc.tensor.matmul(
                        ps, wqk_sb[:, kt, mt * 128 : (mt + 1) * 128],
                        h1T[kt], start=(kt == 0), stop=(kt == 2))
                sb = qks.tile([128, ST], BF16, tag=f"qk{mt}", name=f"qk{mt}")
                nc.scalar.activation(out=sb, in_=ps, func=AF.Copy)
                qk_sb.append(sb)

            v_sb = []
            for rp in range(4):
                ps = qkp.tile([RP, CP], F32, tag="qkps", name="qkps")
                for kt in range(3):
                    nc.tensor.matmul(
                        ps, h1T[kt][:, rp * RP : (rp + 1) * RP],
                        wv_sb[:, kt, :], start=(kt == 0), stop=(kt == 2))
                sb = vsb.tile([RP, CP], BF16, tag="v", name="v")
                nc.scalar.activation(out=sb, in_=ps, func=AF.Copy)
                v_sb.append(sb)

            if debug and st == 0:
                for ct in range(3):
                    nc.gpsimd.dma_start(
                        out=dbg["h1T"].rearrange("(a p) t -> a p t", p=128)[ct],
                        in_=h1T[ct][:, :ST])
                for mt in range(8):
                    nc.gpsimd.dma_start(
                        out=dbg["qk"].rearrange("(a p) t -> a p t", p=128)[mt],
                        in_=qk_sb[mt])
                for rp in range(4):
                    nc.gpsimd.dma_start(
                        out=dbg["v"].rearrange("(a p) d -> a p d", p=RP)[rp],
                        in_=v_sb[rp])
            S["qk_sb"] = qk_sb
            S["v_sb"] = v_sb
            return S

        def back1(st, S):
            qk_sb, v_sb, xpb = S["qk_sb"], S["v_sb"], S["xpb"]
            attn_sb = [ats.tile([128, ST], BF16, tag=f"attn{kt}", name=f"attn{kt}") for kt in range(4)]
            for rp in range(4):
                tsl = slice(rp * RP, (rp + 1) * RP)
                for hp in range(4):
                    aps = atp.tile([128, RP], F32, tag="attnps", name="attnps")
                    for sub in range(2):  # head 2*hp + sub
                        b0 = 64 * sub
                        sps = scp.tile([RP, RP], F32, tag="scores", name="scores")
                        nc.tensor.matmul(
                            sps, qk_sb[hp][b0 : b0 + 64, tsl],
                            qk_sb[4 + hp][b0 : b0 + 64, tsl],
                            start=True, stop=True, tile_position=(b0, 0))
                        ngm = smx.tile([RP, RP], F32, tag="ngm", name="ngm")
                        nc.vector.tensor_tensor(ngm, sps, madd_sb, ALU.add)
                        mx8 = smv.tile([RP, 8], F32, tag="mx8", name="mx8")
                        nc.vector.max(out=mx8, in_=ngm)
                        ngx = smv.tile([RP, 1], F32, tag="ngx", name="ngx")
                        nc.vector.tensor_scalar_mul(ngx, mx8[:, 0:1], -1.0)
                        expw = smx.tile([RP, RP], BF16, tag="expw", name="expw")
                        sums = smv.tile([RP, 1], F32, tag="sums", name="sums")
                        nc.scalar.activation(
                            out=expw, in_=ngm, func=AF.Exp,
                            bias=ngx, scale=1.0, accum_out=sums)
                        rcp = smv.tile([RP, 1], F32, tag="rcp", name="rcp")
                        nc.vector.reciprocal(rcp, sums)
                        wei = smx.tile([RP, RP], BF16, tag="wei", name="wei")
                        nc.gpsimd.tensor_scalar_mul(wei, expw, rcp)
                        weiT = smx.tile([RP, RP], BF16, tag="weiT", name="weiT")
                        nc.sync.dma_start(out=weiT, in_=wei, transpose=True)
                        nc.tensor.matmul(
                            aps[b0 : b0 + 64, :],
                            v_sb[rp][:, 128 * hp + b0 : 128 * hp + b0 + 64],
                            weiT, start=True, stop=True,
                            tile_position=(0, b0))
                    nc.vector.tensor_copy(out=attn_sb[hp][:, tsl], in_=aps)

            if debug and st == 0:
                for kt in range(4):
                    nc.gpsimd.dma_start(
                        out=dbg["attn"].rearrange("(a p) t -> a p t", p=128)[kt],
                        in_=attn_sb[kt])

            x2_t = []
            for rp in range(4):
                tsl = slice(rp * RP, (rp + 1) * RP)
                ps = ffp.tile([RP, C], F32, tag="ffps", name="ffps")
                for kt in range(4):
                    nc.tensor.matmul(
                        ps, attn_sb[kt][:, tsl], wp_sb[:, kt, :],
                        start=(kt == 0), stop=(kt == 3))
                x2 = x2p.tile([RP, C], BF16, tag="x2", name="x2")
                nc.vector.tensor_tensor(out=x2, in0=ps, in1=xpb[rp], op=ALU.add)
                x2_t.append(x2)

            if debug and st == 0:
                for rp in range(4):
                    nc.gpsimd.dma_start(
                        out=dbg["x2"].rearrange("(a p) c -> a p c", p=RP)[rp],
                        in_=x2_t[rp])

            h2 = layer_norm(x2_t, "ln2g", "ln2b")
            h2T = transpose_sb(h2, "h2T")
            S["x2_t"] = x2_t
            S["h2T"] = h2T
            return S

        def back2(st, S):
            h2T, x2_t = S["h2T"], S["x2_t"]
            relu_sb = []
            for mt in range(12):
                ps = ffp.tile([128, ST], F32, tag="ffps", name="ffps")
                for kt in range(3):
                    nc.tensor.matmul(
                        ps, w1_sb[:, kt, mt * 128 : (mt + 1) * 128],
                        h2T[kt], start=(kt == 0), stop=(kt == 2))
                sb = rlu.tile([128, ST], BF16, tag=f"rl{mt}", name=f"rl{mt}")
                nc.scalar.activation(
                    out=sb, in_=ps, func=AF.Relu,
                    bias=(0.0 if trivial_bias else b1_sb[:, mt : mt + 1]))
                relu_sb.append(sb)

            if debug and st == 0:
                for mt in range(12):
                    nc.gpsimd.dma_start(
                        out=dbg["relu"].rearrange("(a p) t -> a p t", p=128)[mt],
                        in_=relu_sb[mt])

            f2_sb = []
            for mt in range(3):
                ps = ffp.tile([128, ST], F32, tag="ffps", name="ffps")
                for kt in range(12):
                    nc.tensor.matmul(
                        ps, w2_sb[:, kt, mt * 128 : (mt + 1) * 128],
                        relu_sb[kt], start=(kt == 0), stop=(kt == 11))
                sb = f2s.tile([128, ST], BF16, tag=f"f2{mt}", name=f"f2{mt}")
                nc.scalar.activation(
                    out=sb, in_=ps, func=AF.Copy,
                    bias=(0.0 if trivial_bias else b2_sb[:, mt : mt + 1]))
                f2_sb.append(sb)

            for rp in range(4):
                tsl = slice(rp * RP, (rp + 1) * RP)
                ot = oub.tile([RP, C], BF16, tag="out", name="out")
                ftp = oub.tile([RP, C], BF16, tag="ftp", name="ftp")
                for ct in range(3):
                    nc.sync.dma_start(
                        out=ftp[:, ct * 128 : (ct + 1) * 128],
                        in_=f2_sb[ct][:, tsl], transpose=True)
                nc.vector.tensor_tensor(out=ot, in0=ftp, in1=x2_t[rp], op=ALU.add)
                nc.gpsimd.dma_start(out=out_v[st * 4 + rp], in_=ot)

        states = {}
        for st in range(n_st + 2):
            if st < n_st:
                states[st] = front_ln(st)
            if 1 <= st <= n_st:
                back1(st - 1, states[st - 1])
            if st < n_st:
                front_mm(st, states[st])
            if st >= 2:
                back2(st - 2, states.pop(st - 2))

    return nc


def ref_shard(x, inputs):
    """Numpy fp32 reference for one shard x [n, C] (n multiple of T)."""
    x = np.asarray(x, np.float32)
    Wq = np.asarray(inputs["Wq"], np.float32)
    Wk = np.asarray(inputs["Wk"], np.float32)
    Wv = np.asarray(inputs["Wv"], np.float32)
    Wp = np.asarray(inputs["Wp"], np.float32)
    W1 = np.asarray(inputs["W1"], np.float32)
    W2 = np.asarray(inputs["W2"], np.float32)
    bp = np.asarray(inputs["bp"], np.float32)
    b1 = np.asarray(inputs["b1"], np.float32)
    b2 = np.asarray(inputs["b2"], np.float32)

    def ln(v, g, b):
        mu = v.mean(-1, keepdims=True)
        var = ((v - mu) ** 2).mean(-1, keepdims=True)
        return (v - mu) / np.sqrt(var + EPS) * g + b

    B = x.shape[0] // T
    xb = x.reshape(B, T, C)
    h = ln(xb, inputs["ln1_g"], inputs["ln1_b"])
    q = np.einsum("btc,hcd->bhtd", h, Wq)
    k = np.einsum("btc,hcd->bhtd", h, Wk)
    v = np.einsum("btc,hcd->bhtd", h, Wv)
    wei = np.einsum("bhtd,bhsd->bhts", q, k) * SCALE
    causal = np.tril(np.ones((T, T), bool))
    wei = np.where(causal, wei, -np.inf)
    wei = wei - wei.max(-1, keepdims=True)
    wei = np.exp(wei)
    wei = wei / wei.sum(-1, keepdims=True)
    attn = np.einsum("bhts,bhsd->bhtd", wei, v)
    attn = attn.transpose(0, 2, 1, 3).reshape(B, T, C)
    xb = attn @ Wp + bp + xb
    h2 = ln(xb, inputs["ln2_g"], inputs["ln2_b"])
    ff = np.maximum(h2 @ W1 + b1, 0.0) @ W2 + b2
    return (ff + xb).reshape(-1, C)


# ---------------------------------------------------------------------------
# BIR post-processing: this walrus build caps sync-wait commands per
# instruction (1 for CTRL-encoded ops, small for others).  Split excess waits
# onto same-engine NoOps inserted immediately before the instruction.
WAIT_LIMITS = {"NoOp": 1, "Drain": 1, "EventSemaphore": 1, "Branch": 1,
               "DmaTransposeAnt": 1}
WAIT_LIMIT_DEFAULT = 1
_wsplit_n = [0]


def fix_bir_json(raw: bytes) -> bytes:
    import orjson
    d = orjson.loads(raw)
    for fn in d["functions"]:
        for bb in fn["blocks"]:
            insts = bb["instructions"]
            out = []
            for inst in insts:
                si = inst.get("sync_info")
                ow = (si or {}).get("on_wait") or []
                lim = WAIT_LIMITS.get(inst.get("opcode"), WAIT_LIMIT_DEFAULT)
                if len(ow) > lim:
                    keep = ow[-lim:] if lim > 0 else []
                    extra = ow[: len(ow) - lim]
                    for w in extra:
                        _wsplit_n[0] += 1
                        out.append({
                            "debug": inst.get("debug", 0),
                            "engine": inst["engine"],
                            "ins": [], "outs": [],
                            "name": f"WSPLIT-{_wsplit_n[0]}",
                            "opcode": "NoOp",
                            "sync_info": {"on_update": [], "on_wait": [w]},
                        })
                    si["on_wait"] = keep
                out.append(inst)
            bb["instructions"] = out
    return orjson.dumps(d)


def wrap_to_json(nc):
    orig = nc.to_json_bytes
    nc.to_json_bytes = lambda: fix_bir_json(orig())
    return nc


# ---------------------------------------------------------------------------
# kernel entry point
N_CORES = 8
_WKEYS = ("wqk", "wv", "wp", "w1", "w2", "madd", "ident", "bp", "b1", "b2")
_CACHE = {}


def _get_nc(n_tok, triv_ln, triv_b):
    key = (n_tok, triv_ln, triv_b)
    if key not in _CACHE:
        _CACHE[key] = wrap_to_json(
            build_nc(n_tok, trivial_ln=triv_ln, trivial_bias=triv_b))
    return _CACHE[key]


def kernel(**inputs):
    from concourse.bass_utils import run_bass_kernel_spmd

    inputs = {k: np.asarray(v) for k, v in inputs.items()}
    x = np.asarray(inputs["x"], np.float32)
    B, T_, C_ = x.shape
    assert C_ == C and B % N_CORES == 0 and (B // N_CORES) * T_ % ST == 0
    n_tok = (B // N_CORES) * T_
    w = build_weights(inputs)
    triv_ln = all(np.allclose(np.asarray(inputs[k], np.float32), v)
                  for k, v in (("ln1_g", 1.0), ("ln1_b", 0.0),
                               ("ln2_g", 1.0), ("ln2_b", 0.0)))
    triv_b = all(np.allclose(np.asarray(inputs[k], np.float32), 0.0)
                 for k in ("bp", "b1", "b2"))
    nc = _get_nc(n_tok, triv_ln, triv_b)
    xs = x.reshape(N_CORES, n_tok, C).astype(np.float16)
    base = {k: w[k] for k in _WKEYS}
    if not triv_ln:
        base.update(ln1g=w["ln1g"], ln1b=w["ln1b"],
                    ln2g=w["ln2g"], ln2b=w["ln2b"])
    in_maps = [dict(base, x=xs[i]) for i in range(N_CORES)]
    res = run_bass_kernel_spmd(nc, in_maps, core_ids=list(range(N_CORES)))
    out = np.stack([res.results[i]["out"] for i in range(N_CORES)])
    return out.reshape(B, T_, C_).astype(np.float32)



# revision 3
# speedup vs baseline: 5.7080x; 5.7080x over previous
"""TRN2 Bass kernel for nn_Block_58574763983799 (dense transformer block).

Self-contained: builds/compiles the Bass program on first call (cached),
shards the batch (data-parallel) over 8 NeuronCores, computes in fp16
(fp32 accumulation in PSUM; layernorm statistics and softmax in fp32),
returns fp32.

  kernel(**inputs) -> np.ndarray [2048, 64, 384] float32
"""
import sys
if "/opt/trn_rl_repo" not in sys.path:
    sys.path.insert(0, "/opt/trn_rl_repo")
import numpy as np
import ml_dtypes

import concourse.bass as bass
import concourse.mybir as mybir
import concourse.tile as tile
from contextlib import ExitStack
from concourse.vector_clock import ScopedClock, VectorClock

F32 = mybir.dt.float32
BF16 = mybir.dt.float16  # 16-bit compute dtype (fp16: need mantissa for softmax exponent accuracy)
AF = mybir.ActivationFunctionType
ALU = mybir.AluOpType

C = 384
H = 8
D = 48
DP = 64          # padded head dim
T = 64           # tokens per row (attention block)
FF = 4 * C       # 1536
CP = H * DP      # 512 padded concat dim
RP = 128         # rowpair tokens
ST = 512         # supertile tokens
NEG = -1.0e9
SCALE = D * (C ** -0.5)
EPS = 1e-5
MAGIC = 0x5F3759DF


def patch_drain():
    """Split the TileContext tail-drain's semaphore waits into 1-wait NOPs.

    This walrus build rejects >2 sync-wait commands on SP CTRL instructions
    ("Too many sync wait commands"), and the stock tail drain carries one wait
    per live processor.  Chain single-wait NOPs on SP instead; program order on
    the SP sequencer then makes the drain itself wait-free.
    """
    if getattr(tile.TileContext, "_drain_patched", False):
        return

    def _drain_and_barrier(self, tick_clock, wait_clock):
        nc = self.nc
        gcv = tick_clock.global_clock
        ticks = list(gcv)
        nz = [(i, t) for i, t in enumerate(ticks) if t > 0]
        for i, t in nz:
            vec = [0] * len(ticks)
            vec[i] = t
            nop = nc.sync.nop()
            wait_clock.add_sem_waits(nop.ins, ScopedClock({None: VectorClock(vec)}))
        nc.sync.drain()
        nc.all_engine_barrier()
        assert self.sems is not None
        popped = nc._tile_sem_poison_stack.pop()
        assert popped is self._sem_poison
        nc.clear_and_free_semaphores(list(self.sems.allocated().values()))
        nc.all_engine_barrier()

    tile.TileContext._drain_and_barrier = _drain_and_barrier
    tile.TileContext._drain_patched = True


def build_weights(inputs):
    """Host-side weight prep from the raw reference inputs (numpy f32)."""
    Wq = np.asarray(inputs["Wq"], np.float32)  # [H, C, D]
    Wk = np.asarray(inputs["Wk"], np.float32)
    Wv = np.asarray(inputs["Wv"], np.float32)
    Wp = np.asarray(inputs["Wp"], np.float32)  # [C, C]
    W1 = np.asarray(inputs["W1"], np.float32)  # [C, FF]
    W2 = np.asarray(inputs["W2"], np.float32)  # [FF, C]

    bf = np.float16
    # wqk [C, 2*CP]: cols 64h..64h+47 = Wq_h * SCALE (scale folded into q)
    wqk = np.zeros((C, 2 * CP), np.float32)
    for h in range(H):
        wqk[:, DP * h : DP * h + D] = Wq[h] * SCALE
        wqk[:, CP + DP * h : CP + DP * h + D] = Wk[h]
    # wv [C, CP]
    wv = np.zeros((C, CP), np.float32)
    for h in range(H):
        wv[:, DP * h : DP * h + D] = Wv[h]
    # wp [CP, C] padded rows
    wp = np.zeros((CP, C), np.float32)
    for h in range(H):
        wp[DP * h : DP * h + D, :] = Wp[D * h : D * h + D, :]

    # block-causal additive mask [RP, RP] (2 diag 64-blocks, causal within)
    madd = np.full((RP, RP), NEG, np.float32)
    for b in range(2):
        for tl in range(T):
            madd[b * T + tl, b * T : b * T + tl + 1] = 0.0

    ident = np.eye(128, dtype=np.float32)

    return {
        "wqk": wqk.astype(bf),
        "wv": wv.astype(bf),
        "wp": wp.astype(bf),
        "w1": W1.astype(bf),
        "w2": W2.astype(bf),
        "madd": madd,
        "ident": ident.astype(bf),
        "bp": np.asarray(inputs["bp"], np.float32),
        "b1": np.asarray(inputs["b1"], np.float32),
        "b2": np.asarray(inputs["b2"], np.float32),
        "ln1g": np.asarray(inputs["ln1_g"], np.float32),
        "ln1b": np.asarray(inputs["ln1_b"], np.float32),
        "ln2g": np.asarray(inputs["ln2_g"], np.float32),
        "ln2b": np.asarray(inputs["ln2_b"], np.float32),
    }


def build_nc(n_tok, trivial_ln=True, trivial_bias=True, debug=False, n_rep=1):
    """Build the Bass program for one core processing [n_tok, C] tokens.

    n_rep > 1 repeats the whole token loop (same data, same output) for
    in-program timing via wall-clock slope between repeat counts."""
    patch_drain()
    assert n_tok % ST == 0
    n_st = n_tok // ST
    nc = bass.Bass()

    x_d = nc.dram_tensor("x", [n_tok, C], BF16, kind="ExternalInput")
    wqk_d = nc.dram_tensor("wqk", [C, 2 * CP], BF16, kind="ExternalInput")
    wv_d = nc.dram_tensor("wv", [C, CP], BF16, kind="ExternalInput")
    wp_d = nc.dram_tensor("wp", [CP, C], BF16, kind="ExternalInput")
    w1_d = nc.dram_tensor("w1", [C, FF], BF16, kind="ExternalInput")
    w2_d = nc.dram_tensor("w2", [FF, C], BF16, kind="ExternalInput")
    madd_d = nc.dram_tensor("madd", [RP, RP], F32, kind="ExternalInput")
    ident_d = nc.dram_tensor("ident", [128, 128], BF16, kind="ExternalInput")
    bp_d = nc.dram_tensor("bp", [C], F32, kind="ExternalInput")
    b1_d = nc.dram_tensor("b1", [FF], F32, kind="ExternalInput")
    b2_d = nc.dram_tensor("b2", [C], F32, kind="ExternalInput")
    if not trivial_ln:
        ln1g_d = nc.dram_tensor("ln1g", [C], F32, kind="ExternalInput")
        ln1b_d = nc.dram_tensor("ln1b", [C], F32, kind="ExternalInput")
        ln2g_d = nc.dram_tensor("ln2g", [C], F32, kind="ExternalInput")
        ln2b_d = nc.dram_tensor("ln2b", [C], F32, kind="ExternalInput")
    out_d = nc.dram_tensor("out", [n_tok, C], BF16, kind="ExternalOutput")
    dbg = {}
    if debug:
        for nm, shp in (("h1T", [C, n_tok]), ("qk", [2 * CP, n_tok]),
                        ("v", [n_tok, CP]), ("wei", [n_tok, RP]),
                        ("attn", [CP, n_tok]), ("x2", [n_tok, C]),
                        ("relu", [FF, n_tok])):
            dbg[nm] = nc.dram_tensor("dbg_" + nm, shp, BF16, kind="ExternalOutput")

    with tile.TileContext(nc) as tc, ExitStack() as ctx:
        cpool = ctx.enter_context(tc.tile_pool(name="consts", bufs=1))

        # ---- constants into SBUF ----
        wqk_sb = cpool.tile([128, 3, 2 * CP], BF16)
        nc.gpsimd.dma_start(out=wqk_sb, in_=wqk_d.rearrange("(a p) n -> p a n", p=128))
        wv_sb = cpool.tile([128, 3, CP], BF16)
        nc.gpsimd.dma_start(out=wv_sb, in_=wv_d.rearrange("(a p) n -> p a n", p=128))
        wp_sb = cpool.tile([128, 4, C], BF16)
        nc.gpsimd.dma_start(out=wp_sb, in_=wp_d.rearrange("(a p) n -> p a n", p=128))
        w1_sb = cpool.tile([128, 3, FF], BF16)
        nc.gpsimd.dma_start(out=w1_sb, in_=w1_d.rearrange("(a p) n -> p a n", p=128))
        w2_sb = cpool.tile([128, 12, C], BF16)
        nc.gpsimd.dma_start(out=w2_sb, in_=w2_d.rearrange("(a p) n -> p a n", p=128))
        madd_sb = cpool.tile([128, RP], F32)
        nc.gpsimd.dma_start(out=madd_sb, in_=madd_d[:, :])
        ident_sb = cpool.tile([128, 128], BF16)
        nc.gpsimd.dma_start(out=ident_sb, in_=ident_d[:, :])
        b1_sb = cpool.tile([128, 12], F32)
        nc.gpsimd.dma_start(out=b1_sb, in_=b1_d.rearrange("(a p) -> p a", p=128))
        b2_sb = cpool.tile([128, 3], F32)
        nc.gpsimd.dma_start(out=b2_sb, in_=b2_d.rearrange("(a p) -> p a", p=128))
        bpb_sb = cpool.tile([128, C], BF16)
        nc.gpsimd.dma_start(out=bpb_sb, in_=bp_d[None, :].to_broadcast([128, C]))
        magic_sb = cpool.tile([128, 4], mybir.dt.uint32)
        nc.vector.memset(magic_sb, MAGIC)
        csub_sb = cpool.tile([128, 1], F32)
        nc.vector.memset(csub_sb, -40.0)
        if not trivial_ln:
            lnb = {}
            for nm, dten in (("ln1g", ln1g_d), ("ln1b", ln1b_d),
                             ("ln2g", ln2g_d), ("ln2b", ln2b_d)):
                t_ = cpool.tile([128, C], BF16, tag=nm)
                nc.gpsimd.dma_start(out=t_, in_=dten[None, :].to_broadcast([128, C]))
                lnb[nm] = t_

        # ---- pools ----
        xin = ctx.enter_context(tc.tile_pool(name="xin", bufs=12))
        stat = ctx.enter_context(tc.tile_pool(name="stat", bufs=4))
        hbuf = ctx.enter_context(tc.tile_pool(name="hbuf", bufs=8))
        htp = ctx.enter_context(tc.tile_pool(name="htp", bufs=2))
        qkp = ctx.enter_context(tc.tile_pool(name="qkp", bufs=3, space="PSUM"))
        ffp = ctx.enter_context(tc.tile_pool(name="ffp", bufs=2, space="PSUM"))
        qks = ctx.enter_context(tc.tile_pool(name="qks", bufs=2))
        vsb = ctx.enter_context(tc.tile_pool(name="vsb", bufs=8))
        scp = ctx.enter_context(tc.tile_pool(name="scp", bufs=2, space="PSUM"))
        smx = ctx.enter_context(tc.tile_pool(name="smx", bufs=6))
        smv = ctx.enter_context(tc.tile_pool(name="smv", bufs=12))
        atp = ctx.enter_context(tc.tile_pool(name="atp", bufs=1, space="PSUM"))
        ats = ctx.enter_context(tc.tile_pool(name="ats", bufs=2))
        x2p = ctx.enter_context(tc.tile_pool(name="x2p", bufs=8))
        rlu = ctx.enter_context(tc.tile_pool(name="rlu", bufs=2))
        f2s = ctx.enter_context(tc.tile_pool(name="f2s", bufs=2))
        oub = ctx.enter_context(tc.tile_pool(name="oub", bufs=8))

        x_v = x_d.rearrange("(s p) c -> s p c", p=RP)      # [n_rp, 128, C]
        out_v = out_d.rearrange("(s p) c -> s p c", p=RP)

        def layer_norm(src_tiles, gname, bname):
            """src_tiles: 4 SBUF [128, C] bf16 tiles -> returns 4 h tiles bf16."""
            mv = stat.tile([128, 4, 2], F32, tag="mv", name="mv")
            for rp in range(4):
                bstat = stat.tile([128, 6], F32, tag="bstat", name="bstat")
                nc.vector.bn_stats(out=bstat, in_=src_tiles[rp])
                nc.vector.bn_aggr(out=mv[:, rp, :], in_=bstat)
            # rstd = rsqrt(var + eps), batched Newton on [128, 4]
            ve = stat.tile([128, 4], F32, tag="ve", name="ve")
            nc.vector.tensor_scalar_add(ve, mv[:, :, 1], EPS)
            vh = stat.tile([128, 4], F32, tag="vh", name="vh")
            nc.vector.tensor_scalar_mul(vh, ve, -0.5)
            ub = stat.tile([128, 4], mybir.dt.uint32, tag="ub", name="ub")
            nc.vector.tensor_scalar(
                out=ub, in0=ve.bitcast(mybir.dt.uint32), scalar1=1,
                scalar2=None, op0=ALU.logical_shift_right)
            y = stat.tile([128, 4], F32, tag="y", name="y")
            nc.vector.tensor_tensor(
                out=y.bitcast(mybir.dt.uint32), in0=magic_sb, in1=ub,
                op=ALU.subtract)
            tq = stat.tile([128, 4], F32, tag="tq", name="tq")
            for _ in range(2):
                nc.vector.tensor_tensor(out=tq, in0=y, in1=y, op=ALU.mult)
                nc.vector.tensor_tensor(out=tq, in0=tq, in1=vh, op=ALU.mult)
                nc.vector.tensor_scalar_add(tq, tq, 1.5)
                nc.vector.tensor_tensor(out=y, in0=y, in1=tq, op=ALU.mult)
            hs = []
            for rp in range(4):
                h_ = hbuf.tile([128, C], BF16, tag="h", name="h")
                nc.vector.tensor_scalar(
                    out=h_, in0=src_tiles[rp],
                    scalar1=mv[:, rp, 0:1], scalar2=y[:, rp : rp + 1],
                    op0=ALU.subtract, op1=ALU.mult)
                if not trivial_ln:
                    nc.vector.tensor_tensor(out=h_, in0=h_, in1=lnb[gname], op=ALU.mult)
                    nc.vector.tensor_tensor(out=h_, in0=h_, in1=lnb[bname], op=ALU.add)
                hs.append(h_)
            return hs

        def transpose_sb(h_tiles, tag):
            """4x [128, C] bf16 -> hT 3x [128, ST] bf16 via DMA-transpose."""
            hT = [htp.tile([128, ST], BF16, tag=f"{tag}{ct}", name=f"{tag}{ct}") for ct in range(3)]
            for rp in range(4):
                for ct in range(3):
                    nc.sync.dma_start(
                        out=hT[ct][:, rp * RP : (rp + 1) * RP],
                        in_=h_tiles[rp][:, ct * 128 : (ct + 1) * 128],
                        transpose=True)
            return hT

        xq = {}

        def load_x(st):
            x_t = []
            for rp in range(4):
                xt = xin.tile([RP, C], BF16, tag="x", name="x")
                nc.gpsimd.dma_start(out=xt, in_=x_v[st * 4 + rp])
                x_t.append(xt)
            xq[st] = x_t

        def front_ln(st):
            if st not in xq:
                load_x(st)
            if st + 1 < n_st:
                load_x(st + 1)
            x_t = xq.pop(st)

            h1 = layer_norm(x_t, "ln1g", "ln1b")
            h1T = transpose_sb(h1, "h1T")

            if trivial_bias:
                xpb = x_t
            else:
                xpb = []
                for rp in range(4):
                    xp_ = xin.tile([RP, C], BF16, tag="xpb", name="xpb")
                    nc.vector.tensor_tensor(out=xp_, in0=x_t[rp], in1=bpb_sb, op=ALU.add)
                    xpb.append(xp_)
            return dict(x_t=x_t, xpb=xpb, h1T=h1T)

        def front_mm(st, S):
            h1T = S["h1T"]
            qk_sb = []
            for mt in range(8):
                ps = qkp.tile([128, ST], F32, tag="qkps", name="qkps")
                for kt in range(3):
                    nc.tensor.matmul(
                        ps, wqk_sb[:, kt, mt * 128 : (mt + 1) * 128],
                        h1T[kt], start=(kt == 0), stop=(kt == 2))
                sb = qks.tile([128, ST], BF16, tag=f"qk{mt}", name=f"qk{mt}")
                nc.scalar.activation(out=sb, in_=ps, func=AF.Copy)
                qk_sb.append(sb)

            v_sb = []
            for rp in range(4):
                ps = qkp.tile([RP, CP], F32, tag="qkps", name="qkps")
                for kt in range(3):
                    nc.tensor.matmul(
                        ps, h1T[kt][:, rp * RP : (rp + 1) * RP],
                        wv_sb[:, kt, :], start=(kt == 0), stop=(kt == 2))
                sb = vsb.tile([RP, CP], BF16, tag="v", name="v")
                nc.scalar.activation(out=sb, in_=ps, func=AF.Copy)
                v_sb.append(sb)

            if debug and st == 0:
                for ct in range(3):
                    nc.gpsimd.dma_start(
                        out=dbg["h1T"].rearrange("(a p) t -> a p t", p=128)[ct],
                        in_=h1T[ct][:, :ST])
                for mt in range(8):
                    nc.gpsimd.dma_start(
                        out=dbg["qk"].rearrange("(a p) t -> a p t", p=128)[mt],
                        in_=qk_sb[mt])
                for rp in range(4):
                    nc.gpsimd.dma_start(
                        out=dbg["v"].rearrange("(a p) d -> a p d", p=RP)[rp],
                        in_=v_sb[rp])
            S["qk_sb"] = qk_sb
            S["v_sb"] = v_sb
            return S

        def back1(st, S):
            qk_sb, v_sb, xpb = S["qk_sb"], S["v_sb"], S["xpb"]
            attn_sb = [ats.tile([128, ST], BF16, tag=f"attn{kt}", name=f"attn{kt}") for kt in range(4)]
            for rp in range(4):
                tsl = slice(rp * RP, (rp + 1) * RP)
                for hp in range(4):
                    aps = atp.tile([128, RP], F32, tag="attnps", name="attnps")
                    for sub in range(2):  # head 2*hp + sub
                        b0 = 64 * sub
                        sps = scp.tile([RP, RP], F32, tag="scores", name="scores")
                        nc.tensor.matmul(
                            sps, qk_sb[hp][b0 : b0 + 64, tsl],
                            qk_sb[4 + hp][b0 : b0 + 64, tsl],
                            start=True, stop=True, tile_position=(b0, 0))
                        ngm = smx.tile([RP, RP], F32, tag="ngm", name="ngm")
                        nc.vector.tensor_tensor(ngm, sps, madd_sb, ALU.add)
                        mx8 = smv.tile([RP, 8], F32, tag="mx8", name="mx8")
                        nc.vector.max(out=mx8, in_=ngm)
                        ngx = smv.tile([RP, 1], F32, tag="ngx", name="ngx")
                        nc.vector.tensor_scalar_mul(ngx, mx8[:, 0:1], -1.0)
                        expw = smx.tile([RP, RP], BF16, tag="expw", name="expw")
                        sums = smv.tile([RP, 1], F32, tag="sums", name="sums")
                        nc.scalar.activation(
                            out=expw, in_=ngm, func=AF.Exp,
                            bias=ngx, scale=1.0, accum_out=sums)
                        rcp = smv.tile([RP, 1], F32, tag="rcp", name="rcp")
                        nc.vector.reciprocal(rcp, sums)
                        wei = smx.tile([RP, RP], BF16, tag="wei", name="wei")
                        nc.gpsimd.tensor_scalar_mul(wei, expw, rcp)
                        weiT = smx.tile([RP, RP], BF16, tag="weiT", name="weiT")
                        nc.sync.dma_start(out=weiT, in_=wei, transpose=True)
                        nc.tensor.matmul(
                            aps[b0 : b0 + 64, :],
                            v_sb[rp][:, 128 * hp + b0 : 128 * hp + b0 + 64],
                            weiT, start=True, stop=True,
                            tile_position=(0, b0))
                    nc.vector.tensor_copy(out=attn_sb[hp][:, tsl], in_=aps)

            if debug and st == 0:
                for kt in range(4):
                    nc.gpsimd.dma_start(
                        out=dbg["attn"].rearrange("(a p) t -> a p t", p=128)[kt],
                        in_=attn_sb[kt])

            x2_t = []
            for rp in range(4):
                tsl = slice(rp * RP, (rp + 1) * RP)
                ps = ffp.tile([RP, C], F32, tag="ffps", name="ffps")
                for kt in range(4):
                    nc.tensor.matmul(
                        ps, attn_sb[kt][:, tsl], wp_sb[:, kt, :],
                        start=(kt == 0), stop=(kt == 3))
                x2 = x2p.tile([RP, C], BF16, tag="x2", name="x2")
                nc.vector.tensor_tensor(out=x2, in0=ps, in1=xpb[rp], op=ALU.add)
                x2_t.append(x2)

            if debug and st == 0:
                for rp in range(4):
                    nc.gpsimd.dma_start(
                        out=dbg["x2"].rearrange("(a p) c -> a p c", p=RP)[rp],
                        in_=x2_t[rp])

            h2 = layer_norm(x2_t, "ln2g", "ln2b")
            h2T = transpose_sb(h2, "h2T")
            S["x2_t"] = x2_t
            S["h2T"] = h2T
            return S

        def back2(st, S):
            h2T, x2_t = S["h2T"], S["x2_t"]
            relu_sb = []
            for mt in range(12):
                ps = ffp.tile([128, ST], F32, tag="ffps", name="ffps")
                for kt in range(3):
                    nc.tensor.matmul(
                        ps, w1_sb[:, kt, mt * 128 : (mt + 1) * 128],
                        h2T[kt], start=(kt == 0), stop=(kt == 2))
                sb = rlu.tile([128, ST], BF16, tag=f"rl{mt}", name=f"rl{mt}")
                nc.scalar.activation(
                    out=sb, in_=ps, func=AF.Relu,
                    bias=(0.0 if trivial_bias else b1_sb[:, mt : mt + 1]))
                relu_sb.append(sb)

            if debug and st == 0:
                for mt in range(12):
                    nc.gpsimd.dma_start(
                        out=dbg["relu"].rearrange("(a p) t -> a p t", p=128)[mt],
                        in_=relu_sb[mt])

            f2_sb = []
            for mt in range(3):
                ps = ffp.tile([128, ST], F32, tag="ffps", name="ffps")
                for kt in range(12):
                    nc.tensor.matmul(
                        ps, w2_sb[:, kt, mt * 128 : (mt + 1) * 128],
                        relu_sb[kt], start=(kt == 0), stop=(kt == 11))
                sb = f2s.tile([128, ST], BF16, tag=f"f2{mt}", name=f"f2{mt}")
                nc.scalar.activation(
                    out=sb, in_=ps, func=AF.Copy,
                    bias=(0.0 if trivial_bias else b2_sb[:, mt : mt + 1]))
                f2_sb.append(sb)

            for rp in range(4):
                tsl = slice(rp * RP, (rp + 1) * RP)
                ot = oub.tile([RP, C], BF16, tag="out", name="out")
                ftp = oub.tile([RP, C], BF16, tag="ftp", name="ftp")
                for ct in range(3):
                    nc.sync.dma_start(
                        out=ftp[:, ct * 128 : (ct + 1) * 128],
                        in_=f2_sb[ct][:, tsl], transpose=True)
                nc.vector.tensor_tensor(out=ot, in0=ftp, in1=x2_t[rp], op=ALU.add)
                nc.gpsimd.dma_start(out=out_v[st * 4 + rp], in_=ot)

        for _rep in range(n_rep):
            states = {}
            for st in range(n_st + 2):
                if st < n_st:
                    states[st] = front_ln(st)
                if 1 <= st <= n_st:
                    back1(st - 1, states[st - 1])
                if st < n_st:
                    front_mm(st, states[st])
                if st >= 2:
                    back2(st - 2, states.pop(st - 2))

    return nc


def ref_shard(x, inputs):
    """Numpy fp32 reference for one shard x [n, C] (n multiple of T)."""
    x = np.asarray(x, np.float32)
    Wq = np.asarray(inputs["Wq"], np.float32)
    Wk = np.asarray(inputs["Wk"], np.float32)
    Wv = np.asarray(inputs["Wv"], np.float32)
    Wp = np.asarray(inputs["Wp"], np.float32)
    W1 = np.asarray(inputs["W1"], np.float32)
    W2 = np.asarray(inputs["W2"], np.float32)
    bp = np.asarray(inputs["bp"], np.float32)
    b1 = np.asarray(inputs["b1"], np.float32)
    b2 = np.asarray(inputs["b2"], np.float32)

    def ln(v, g, b):
        mu = v.mean(-1, keepdims=True)
        var = ((v - mu) ** 2).mean(-1, keepdims=True)
        return (v - mu) / np.sqrt(var + EPS) * g + b

    B = x.shape[0] // T
    xb = x.reshape(B, T, C)
    h = ln(xb, inputs["ln1_g"], inputs["ln1_b"])
    q = np.einsum("btc,hcd->bhtd", h, Wq)
    k = np.einsum("btc,hcd->bhtd", h, Wk)
    v = np.einsum("btc,hcd->bhtd", h, Wv)
    wei = np.einsum("bhtd,bhsd->bhts", q, k) * SCALE
    causal = np.tril(np.ones((T, T), bool))
    wei = np.where(causal, wei, -np.inf)
    wei = wei - wei.max(-1, keepdims=True)
    wei = np.exp(wei)
    wei = wei / wei.sum(-1, keepdims=True)
    attn = np.einsum("bhts,bhsd->bhtd", wei, v)
    attn = attn.transpose(0, 2, 1, 3).reshape(B, T, C)
    xb = attn @ Wp + bp + xb
    h2 = ln(xb, inputs["ln2_g"], inputs["ln2_b"])
    ff = np.maximum(h2 @ W1 + b1, 0.0) @ W2 + b2
    return (ff + xb).reshape(-1, C)


# ---------------------------------------------------------------------------
# BIR post-processing: this walrus build caps sync-wait commands per
# instruction (1 for CTRL-encoded ops, small for others).  Split excess waits
# onto same-engine NoOps inserted immediately before the instruction.
WAIT_LIMITS = {"NoOp": 1, "Drain": 1, "EventSemaphore": 1, "Branch": 1,
               "DmaTransposeAnt": 1}
WAIT_LIMIT_DEFAULT = 1
_wsplit_n = [0]


def fix_bir_json(raw: bytes) -> bytes:
    import orjson
    d = orjson.loads(raw)
    for fn in d["functions"]:
        for bb in fn["blocks"]:
            insts = bb["instructions"]
            out = []
            for inst in insts:
                si = inst.get("sync_info")
                ow = (si or {}).get("on_wait") or []
                lim = WAIT_LIMITS.get(inst.get("opcode"), WAIT_LIMIT_DEFAULT)
                if len(ow) > lim:
                    keep = ow[-lim:] if lim > 0 else []
                    extra = ow[: len(ow) - lim]
                    for w in extra:
                        _wsplit_n[0] += 1
                        out.append({
                            "debug": inst.get("debug", 0),
                            "engine": inst["engine"],
                            "ins": [], "outs": [],
                            "name": f"WSPLIT-{_wsplit_n[0]}",
                            "opcode": "NoOp",
                            "sync_info": {"on_update": [], "on_wait": [w]},
                        })
                    si["on_wait"] = keep
                out.append(inst)
            bb["instructions"] = out
    return orjson.dumps(d)


def wrap_to_json(nc):
    orig = nc.to_json_bytes
    nc.to_json_bytes = lambda: fix_bir_json(orig())
    return nc


# ---------------------------------------------------------------------------
# kernel entry point
N_CORES = 8
_WKEYS = ("wqk", "wv", "wp", "w1", "w2", "madd", "ident", "bp", "b1", "b2")
_CACHE = {}


def _get_nc(n_tok, triv_ln, triv_b):
    key = (n_tok, triv_ln, triv_b)
    if key not in _CACHE:
        _CACHE[key] = wrap_to_json(
            build_nc(n_tok, trivial_ln=triv_ln, trivial_bias=triv_b))
    return _CACHE[key]


def kernel(**inputs):
    from concourse.bass_utils import run_bass_kernel_spmd

    inputs = {k: np.asarray(v) for k, v in inputs.items()}
    x = np.asarray(inputs["x"], np.float32)
    B, T_, C_ = x.shape
    assert C_ == C and B % N_CORES == 0 and (B // N_CORES) * T_ % ST == 0
    n_tok = (B // N_CORES) * T_
    w = build_weights(inputs)
    triv_ln = all(np.allclose(np.asarray(inputs[k], np.float32), v)
                  for k, v in (("ln1_g", 1.0), ("ln1_b", 0.0),
                               ("ln2_g", 1.0), ("ln2_b", 0.0)))
    triv_b = all(np.allclose(np.asarray(inputs[k], np.float32), 0.0)
                 for k in ("bp", "b1", "b2"))
    nc = _get_nc(n_tok, triv_ln, triv_b)
    xs = x.reshape(N_CORES, n_tok, C).astype(np.float16)
    base = {k: w[k] for k in _WKEYS}
    if not triv_ln:
        base.update(ln1g=w["ln1g"], ln1b=w["ln1b"],
                    ln2g=w["ln2g"], ln2b=w["ln2b"])
    in_maps = [dict(base, x=xs[i]) for i in range(N_CORES)]
    res = run_bass_kernel_spmd(nc, in_maps, core_ids=list(range(N_CORES)))
    out = np.stack([res.results[i]["out"] for i in range(N_CORES)])
    return out.reshape(B, T_, C_).astype(np.float32)

